# revision 1
# baseline (speedup 1.0000x reference)
"""H2GCNConv Trainium2 kernel: dual-path SpMM (DMA gather + GPSIMD ap_gather).

Paths (per core, node-sharded rows, no collectives):
  DMA path (cols >= THRESH): per-edge dma_gather (SWDGE, 256B fp16 rows) into
    degree-sorted 128-row blocks; per tile scale (DVE/ACT) + identity matmul
    accumulate in PSUM. idx = col - THRESH >= 0, so no trailing-negative pad
    tiles are needed.
  Pool path (cols < THRESH): x packed as uint32 feature-pairs in SBUF
    ([128, NE]: partitions 0-63 = nodes [0,NE), 64-127 = nodes [NE,2NE)),
    gpsimd.ap_gather pulls 2 edges per column (low/high half lockstep).
    vals broadcast via 2-contraction PE matmul into PSUM, DVE multiply,
    then interleaved strided adds (block j = j-th edge of each row) into an
    SBUF accumulator. Host adds the two paths' partial outputs.
"""

import sys
from contextlib import ExitStack

import numpy as np

sys.path.insert(0, "/opt/trn_rl_repo")

N_NODES = 50000
D_FEAT = 128
N_CORES = 8
RPC = N_NODES // N_CORES  # 6250 rows per core
NBLK = (RPC + 127) // 128  # 49 blocks (6272 padded rows)
THRESH = 10240  # col split (DVE-balance optimum)
DBIAS = (THRESH + N_NODES) // 2  # signed-idx mid-bias for the DMA path
NE = THRESH // 2  # nodes per table half
CH = 4352  # ap_gather columns per call (>= NE keeps idx-bound cost; %256==0)
GSEG = 8  # tiles (of 128 idx) per dma_gather; 1024 idx max (device limit)
NUM_QUEUES = 2
SCRATCH = 57344


def _block_groups(nblk):
    grps = []
    lo, hi = 0, nblk - 1
    while lo < hi:
        grps.append((lo, hi))
        lo += 1
        hi -= 1
    if lo == hi:
        grps.append((lo,))
    return grps


def _prep_hop_dma(row, col, vals, rpc, n_cores, nblk):
    """DMA-path slot assignment (cols >= THRESH). Same scheme as before:
    per-core degree-desc row sort, blocks of 128 rows, slot (p, t) = t-th
    edge of block's p-th row; T_b = max tiles per block over cores."""
    core = row // rpc
    lrow = row % rpc
    key = core * rpc + lrow

    deg = np.bincount(key, minlength=n_cores * rpc)
    order = np.argsort(key, kind="stable")
    starts = np.zeros(n_cores * rpc + 1, dtype=np.int64)
    starts[1:] = np.cumsum(deg)
    rank = np.empty(len(row), dtype=np.int64)
    rank[order] = np.arange(len(row)) - starts[key[order]]

    degs = deg.reshape(n_cores, rpc)
    perm = np.argsort(-degs, axis=1, kind="stable")
    inv = np.empty_like(perm)
    np.put_along_axis(inv, perm, np.arange(rpc)[None, :].repeat(n_cores, 0), axis=1)
    s_pos = inv[core, lrow]

    degs_sorted = np.take_along_axis(degs, perm, axis=1)
    pad = np.zeros((n_cores, nblk * 128 - rpc), dtype=degs_sorted.dtype)
    degs_sorted = np.concatenate([degs_sorted, pad], axis=1)
    T_b = degs_sorted.reshape(n_cores, nblk, 128).max(axis=(0, 2))

    b = s_pos // 128
    p = s_pos % 128
    return dict(core=core, col=col, vals=vals, b=b, p=p, t=rank, T_b=T_b,
                perm=perm)


def _prep_hop_pool(row, col, vals, rpc, n_cores):
    """Pool-path structures (cols < THRESH). Lockstep halves A (col<NE) and
    B (col in [NE, 2NE)): per row m_r = max(dA, dB) column slots; rows sorted
    by m desc; column (j, s) = j-th edge of sorted row s. R_j shared across
    cores (max)."""
    core = (row // rpc).astype(np.int64)
    lrow = (row % rpc).astype(np.int64)
    half = (col >= NE).astype(np.int64)

    # per (core,row,half) degree + rank
    key = (core * rpc + lrow) * 2 + half
    deg = np.bincount(key, minlength=n_cores * rpc * 2)
    order = np.argsort(key, kind="stable")
    starts = np.zeros(n_cores * rpc * 2 + 1, dtype=np.int64)
    starts[1:] = np.cumsum(deg)
    rank = np.empty(len(row), dtype=np.int64)
    rank[order] = np.arange(len(row)) - starts[key[order]]

    dAB = deg.reshape(n_cores, rpc, 2)
    m = dAB.max(axis=2)  # [c, rpc]
    perm = np.argsort(-m, axis=1, kind="stable")  # s -> lrow
    inv = np.empty_like(perm)
    np.put_along_axis(inv, perm, np.arange(rpc)[None, :].repeat(n_cores, 0), axis=1)
    s_pos = inv[core, lrow]

    m_sorted = np.take_along_axis(m, perm, axis=1)  # desc per core
    jmax = int(m_sorted.max()) if len(row) else 0
    # R_j = max over cores of #{rows with m > j}
    R = np.zeros(jmax, dtype=np.int64)
    for j in range(jmax):
        R[j] = int((m_sorted > j).sum(axis=1).max())
    off = np.zeros(jmax + 1, dtype=np.int64)
    off[1:] = np.cumsum(R)
    C_real = int(off[-1])
    C_pad = -(-max(C_real, 1) // 256) * 256  # %256 for integer 512-col pieces

    # per-edge column position
    pos = off[rank] + s_pos

    idxA = np.zeros((n_cores, C_pad), dtype=np.int16)
    idxB = np.zeros((n_cores, C_pad), dtype=np.int16)
    valA = np.zeros((n_cores, C_pad), dtype=np.float16)
    valB = np.zeros((n_cores, C_pad), dtype=np.float16)
    lo = half == 0
    idxA[core[lo], pos[lo]] = col[lo].astype(np.int16)
    valA[core[lo], pos[lo]] = vals[lo].astype(np.float16)
    hi = ~lo
    idxB[core[hi], pos[hi]] = (col[hi] - NE).astype(np.int16)
    valB[core[hi], pos[hi]] = vals[hi].astype(np.float16)

    # add-piece list: (acc_col_start(sorted-row units), global col range a:b)
    blocks = [(int(off[j]), int(off[j + 1])) for j in range(jmax)]
    if C_pad > C_real:
        blocks.append((C_real, C_pad))  # tail pads: add zeros to acc prefix
    return dict(idxA=idxA, idxB=idxB, valA=valA, valB=valB, blocks=blocks,
                C=C_pad, perm=perm, off=off)


def _prep(x, row1, col1, vals1, row2, col2, vals2):
    x = np.asarray(x).astype(np.float32)
    x16 = x.astype(np.float16)

    # pool-path table: packed fp16 pairs as uint32.
    # xpair[p, c] = pack(x16[c + (p>=64)*NE, 2*(p%64) : 2*(p%64)+2])
    xr = x16[: 2 * NE].reshape(2, NE, 64, 2)  # [half, node, pair, 2]
    xpair = np.ascontiguousarray(
        xr.transpose(0, 2, 1, 3).reshape(128, NE, 2)
    ).view(np.uint32)[:, :, 0]
    xpair = np.ascontiguousarray(xpair)  # [128, NE] uint32

    ones2 = np.zeros((128, 128), dtype=np.float16)
    ones2[0, :64] = 1.0  # lhsT row 0 -> psum partitions 0-63
    ones2[1, 64:] = 1.0  # lhsT row 1 -> psum partitions 64-127

    hops_d = []
    hops_p = []
    for (row, col, vals) in ((row1, col1, vals1), (row2, col2, vals2)):
        row = np.asarray(row).astype(np.int64)
        col = np.asarray(col).astype(np.int64)
        vals = np.asarray(vals).astype(np.float32)
        dm = col >= THRESH
        hops_d.append(_prep_hop_dma(row[dm], col[dm], vals[dm], RPC, N_CORES,
                                    NBLK))
        pm = ~dm
        hops_p.append(_prep_hop_pool(row[pm], col[pm], vals[pm], RPC, N_CORES))

    grps = _block_groups(NBLK)

    # ---- DMA-path idx/vals flat layout (per hop, group-major, no pad tiles)
    tile_off = []  # [hop][block] -> tile index within hop enumeration
    hop_tiles = []
    batches = []  # (h, grp, nt, idx_col_off)
    Wtot = 0
    for h in range(2):
        off = np.zeros(NBLK, dtype=np.int64)
        c = 0
        for grp in grps:
            nt = int(sum(hops_d[h]["T_b"][b] for b in grp))
            for b in grp:
                off[b] = c
                c += int(hops_d[h]["T_b"][b])
            batches.append((h, grp, nt, Wtot))
            Wtot += nt * 8
        tile_off.append(off)
        hop_tiles.append(c)
    T_total = hop_tiles[0] + hop_tiles[1]
    hop_base = [0, hop_tiles[0]]

    # group-local tile position of (h, b, t)
    gpos = []  # [hop][block] -> group-local tile offset
    batch_of_block = [dict(), dict()]
    for h in range(2):
        gp = np.zeros(NBLK, dtype=np.int64)
        for bi, (bh, grp, nt, coff) in enumerate(batches):
            if bh != h:
                continue
            c = 0
            for b in grp:
                gp[b] = c
                c += int(hops_d[h]["T_b"][b])
                batch_of_block[h][b] = bi
        gpos.append(gp)

    idx_flat = np.zeros((N_CORES, Wtot * 16), dtype=np.int16)
    vals_arr = np.zeros((N_CORES, 128, max(T_total, 1)), dtype=np.float32)
    for h in range(2):
        hp = hops_d[h]
        gidx = tile_off[h][hp["b"]] + hp["t"] + hop_base[h]
        vals_arr[hp["core"], hp["p"], gidx] = hp["vals"]
        bi_arr = np.array([batch_of_block[h].get(b, 0) for b in range(NBLK)],
                          dtype=np.int64)
        coffs = np.array([batches[i][3] for i in range(len(batches))],
                         dtype=np.int64)
        flat_pos = (coffs[bi_arr[hp["b"]]] * 16
                    + (gpos[h][hp["b"]] + hp["t"]) * 128 + hp["p"])
        idx_flat[hp["core"], flat_pos] = (hp["col"] - DBIAS).astype(np.int16)

    # Trailing-negative fix: the last wrap index of every gather segment is
    # (last tile, partition 127). Ensure it is >= 0 by (a) swapping a row with
    # non-negative entries at all segment-end t positions into row-slot 127,
    # (b) reordering that row's own edges so segment ends hold nonneg idx.
    swaps = [[dict() for _ in range(2)] for _ in range(N_CORES)]
    for h in range(2):
        hp = hops_d[h]
        for bi, (bh, grp, nt, coff) in enumerate(batches):
            if bh != h or nt == 0:
                continue
            for b in grp:
                tb = int(hops_d[h]["T_b"][b])
                if tb == 0:
                    continue
                g0 = int(gpos[h][b])
                seg_ends = []
                # segment boundaries are group-local (tiles [0, nt) chopped
                # by GSEG); a block-tile is a segment end if its group-local
                # tile index is one before a boundary or the block's last
                for t in range(tb):
                    gt = g0 + t
                    if (gt % GSEG == GSEG - 1) or (gt == nt - 1):
                        seg_ends.append(t)
                if not seg_ends:
                    continue
                for c in range(N_CORES):
                    base = coff * 16
                    colpos = lambda t, q: base + (g0 + t) * 128 + q
                    col_idx = idx_flat[c]
                    # candidate rows: prefer ones already fine at slot 127
                    chosen = None
                    for q in range(127, -1, -1):
                        vals_q = col_idx[[colpos(t, q) for t in seg_ends]]
                        if (vals_q >= 0).all():
                            chosen = q
                            break
                    if chosen is None:
                        # reorder some row's edges: pick row with most nonneg
                        best, bestq = -1, 127
                        for q in range(128):
                            nn = sum(col_idx[colpos(t, q)] >= 0
                                     for t in range(tb))
                            if nn > best:
                                best, bestq = nn, q
                        q = bestq
                        pos_all = [colpos(t, q) for t in range(tb)]
                        vv = col_idx[pos_all].copy()
                        va = vals_arr[c, q, hop_base[h] + tile_off[h][b]:
                                      hop_base[h] + tile_off[h][b] + tb].copy()
                        order = sorted(range(tb),
                                       key=lambda t: 0 if vv[t] >= 0 else 1)
                        # place nonneg at seg_ends first
                        newv = np.empty_like(vv)
                        newa = np.empty_like(va)
                        rest = [t for t in range(tb) if t not in seg_ends]
                        tgt = seg_ends + rest
                        for k, t in enumerate(tgt):
                            newv[t] = vv[order[k]]
                            newa[t] = va[order[k]]
                        col_idx[pos_all] = newv
                        vals_arr[c, q, hop_base[h] + tile_off[h][b]:
                                 hop_base[h] + tile_off[h][b] + tb] = newa
                        chosen = q
                    if chosen != 127:
                        # swap rows chosen <-> 127 across all tiles + vals
                        pa = [colpos(t, chosen) for t in range(tb)]
                        pb = [colpos(t, 127) for t in range(tb)]
                        tmp = col_idx[pa].copy()
                        col_idx[pa] = col_idx[pb]
                        col_idx[pb] = tmp
                        sl = slice(hop_base[h] + tile_off[h][b],
                                   hop_base[h] + tile_off[h][b] + tb)
                        tmpv = vals_arr[c, chosen, sl].copy()
                        vals_arr[c, chosen, sl] = vals_arr[c, 127, sl]
                        vals_arr[c, 127, sl] = tmpv
                        swaps[c][h][b] = chosen

    idx_wrapped = np.zeros((N_CORES, 128, Wtot), dtype=np.int16)
    w = idx_flat.reshape(N_CORES, Wtot, 16).transpose(0, 2, 1)
    idx_wrapped[:] = np.tile(w, (1, 8, 1))

    # ---- pool-path idx (wrapped, halves on partition halves) + vals rows
    C1, C2 = hops_p[0]["C"], hops_p[1]["C"]
    Wp = (C1 + C2) // 16
    pidx = np.zeros((N_CORES, 128, Wp), dtype=np.int16)
    v2w = -(-2 * (C1 + C2) // 4096) * 4096
    vals2 = np.zeros((N_CORES, 2, max(v2w, 4096)), dtype=np.float16)
    for h in range(2):
        hp = hops_p[h]
        C = hp["C"]
        o = 0 if h == 0 else C1 // 16
        vo = 0 if h == 0 else 2 * C1
        wA = hp["idxA"].reshape(N_CORES, C // 16, 16).transpose(0, 2, 1)
        wB = hp["idxB"].reshape(N_CORES, C // 16, 16).transpose(0, 2, 1)
        pidx[:, 0:64, o:o + C // 16] = np.tile(wA, (1, 4, 1))
        pidx[:, 64:128, o:o + C // 16] = np.tile(wB, (1, 4, 1))
        # vals duplicated x2 (uint32 col -> 2 fp16 cols)
        vals2[:, 0, vo:vo + 2 * C:2] = hp["valA"]
        vals2[:, 0, vo + 1:vo + 2 * C:2] = hp["valA"]
        vals2[:, 1, vo:vo + 2 * C:2] = hp["valB"]
        vals2[:, 1, vo + 1:vo + 2 * C:2] = hp["valB"]

    cfg = dict(
        grps=grps, batches=batches, tile_off=tile_off, hop_base=hop_base,
        T_b=[hops_d[0]["T_b"], hops_d[1]["T_b"]], Wtot=Wtot, T_total=T_total,
        gpos=gpos, pool=dict(
            C=[C1, C2], blocks=[hops_p[0]["blocks"], hops_p[1]["blocks"]],
            Wp=Wp,
        ),
    )
    ident = np.eye(128, dtype=np.float16)
    in_maps = []
    for c in range(N_CORES):
        in_maps.append({
            "x16": x16,
            "idxs": idx_wrapped[c],
            "valsbuf": vals_arr[c],
            "ident": ident,
            "xpair": xpair,
            "pidx": pidx[c],
            "vals2": vals2[c],
            "ones2": ones2,
        })
    perms_d = [hops_d[0]["perm"].copy(), hops_d[1]["perm"].copy()]
    for c in range(N_CORES):
        for h in range(2):
            for b, q in swaps[c][h].items():
                s1, s2 = b * 128 + q, b * 128 + 127
                if s1 < RPC and s2 < RPC:
                    pm = perms_d[h][c]
                    pm[s1], pm[s2] = pm[s2], pm[s1]
    perms_p = [hops_p[0]["perm"], hops_p[1]["perm"]]
    return cfg, in_maps, perms_d, perms_p


def _build(cfg, debug=False):
    import concourse.bacc as bacc
    import concourse.bass as bass
    import concourse.mybir as mybir
    import concourse.tile as tile

    f16 = mybir.dt.float16
    f32 = mybir.dt.float32
    i16 = mybir.dt.int16
    u32 = mybir.dt.uint32

    grps = cfg["grps"]
    batches = cfg["batches"]
    T_b = cfg["T_b"]
    tile_off = cfg["tile_off"]
    hop_base = cfg["hop_base"]
    pool_cfg = cfg["pool"]

    nc = bacc.Bacc("TRN2", target_bir_lowering=False, debug=debug,
                   num_devices=N_CORES, num_swdge_queues=NUM_QUEUES,
                   dynamic_dma_scratch_size=SCRATCH)

    x16 = nc.dram_tensor("x16", [N_NODES, D_FEAT], f16, kind="ExternalInput")
    idxs = nc.dram_tensor("idxs", [128, max(cfg["Wtot"], 1)], i16,
                          kind="ExternalInput")
    valsb = nc.dram_tensor("valsbuf", [128, max(cfg["T_total"], 1)], f32,
                           kind="ExternalInput")
    identd = nc.dram_tensor("ident", [128, 128], f16, kind="ExternalInput")
    xpaird = nc.dram_tensor("xpair", [128, NE], u32, kind="ExternalInput")
    pidxd = nc.dram_tensor("pidx", [128, max(pool_cfg["Wp"], 1)], i16,
                           kind="ExternalInput")
    v2w = -(-2 * (pool_cfg["C"][0] + pool_cfg["C"][1]) // 4096) * 4096
    vals2d = nc.dram_tensor("vals2", [2, max(v2w, 4096)], f16,
                            kind="ExternalInput")
    ones2d = nc.dram_tensor("ones2", [128, 128], f16, kind="ExternalInput")
    outs = [
        nc.dram_tensor(f"out{h+1}", [NBLK * 128, D_FEAT], f32,
                       kind="ExternalOutput")
        for h in range(2)
    ]
    pouts = [
        nc.dram_tensor(f"pout{h+1}", [128, 2 * RPC], f16,
                       kind="ExternalOutput")
        for h in range(2)
    ]

    x_src = x16[DBIAS:, :]

    with tile.TileContext(nc) as tc, ExitStack() as ctx:
        const_pool = ctx.enter_context(tc.tile_pool(name="const", bufs=1))
        idx_pool = ctx.enter_context(tc.tile_pool(name="idx", bufs=4))
        g_pool = ctx.enter_context(tc.tile_pool(name="gath", bufs=8))
        sc_pool = ctx.enter_context(tc.tile_pool(name="scaled", bufs=12))
        ps_pool = ctx.enter_context(tc.tile_pool(name="psum", bufs=4,
                                                 space="PSUM"))
        st_pool = ctx.enter_context(tc.tile_pool(name="stage", bufs=6))
        pidx_pool = ctx.enter_context(tc.tile_pool(name="pidx", bufs=3))
        pg_pool = ctx.enter_context(tc.tile_pool(name="pgath", bufs=2))
        vs_pool = ctx.enter_context(tc.tile_pool(name="vslab", bufs=4))
        vps_pool = ctx.enter_context(tc.tile_pool(name="vpsum", bufs=2,
                                                  space="PSUM"))
        acc_pool = ctx.enter_context(tc.tile_pool(name="acc", bufs=1))

        ident_sb = const_pool.tile([128, 128], f16)
        nc.sync.dma_start(ident_sb[:, :], identd[:, :])
        ones2_sb = const_pool.tile([128, 128], f16)
        nc.sync.dma_start(ones2_sb[:, :], ones2d[:, :])
        vals_sb = const_pool.tile([128, max(cfg["T_total"], 1)], f32)
        nc.sync.dma_start(vals_sb[:, :], valsb[:, :])
        xpair_sb = const_pool.tile([128, NE], u32)
        nc.sync.dma_start(xpair_sb[:, :], xpaird[:, :])

        acc = acc_pool.tile([128, 2 * RPC], f16)

        eng_flip = 0
        qn = 0

        idx_tiles = {}
        pidx_tiles = {}

        def emit_dma_idx(h, grp):
            bi = None
            for i, (bh, g, nt, coff) in enumerate(batches):
                if bh == h and g == grp:
                    bi = i
                    break
            _, _, nt, coff = batches[bi]
            if nt == 0:
                return
            w = nt * 8
            it = idx_pool.tile([128, w], i16, tag="idx")
            nc.sync.dma_start(it[:, :], idxs[:, coff:coff + w])
            idx_tiles[(h, grp)] = it

        def emit_dma_group(h, grp):
            nonlocal eng_flip, qn
            bi = None
            for i, (bh, g, nt, coff) in enumerate(batches):
                if bh == h and g == grp:
                    bi = i
                    break
            _, _, nt, coff = batches[bi]
            if nt == 0:
                return
            it = idx_tiles.pop((h, grp))
            segs = {}
            a = 0
            while a < nt:
                b_end = min(a + GSEG, nt)
                nidx = (b_end - a) * 128
                dst = g_pool.tile([128, b_end - a, 128], f16, tag="gath")
                nc.gpsimd.dma_gather(dst[:, :, :], x_src,
                                     it[:, a * 8:b_end * 8], nidx, nidx,
                                     128, queue_num=qn % NUM_QUEUES)
                qn += 1
                segs[a // GSEG] = (dst, a)
                a = b_end
            for b in grp:
                tb = int(T_b[h][b])
                if tb == 0:
                    continue
                psum = ps_pool.tile([128, 128], f32)
                if globals().get("_SKIP_DMACOMPUTE"):
                    continue
                for t in range(tb):
                    g_ = hop_base[h] + tile_off[h][b] + t
                    sc = sc_pool.tile([128, 128], f16)
                    gt_ = cfg["gpos"][h][b] + t
                    dst, seg_a = segs[gt_ // GSEG]
                    src_ap = dst[:, gt_ - seg_a, :]
                    vap = vals_sb[:, g_:g_ + 1]
                    if eng_flip % 20 < 18:
                        nc.vector.tensor_scalar_mul(sc[:, :], src_ap, vap)
                    else:
                        nc.scalar.mul(sc[:, :], src_ap, vap)
                    eng_flip += 1
                    nc.tensor.matmul(psum[:, :], ident_sb[:, :],
                                     sc[:, :], start=(t == 0),
                                     stop=(t == tb - 1))
                stage = st_pool.tile([128, 128], f32)
                nc.scalar.copy(stage[:, :], psum[:, :])
                nc.sync.dma_start(outs[h][b * 128:(b + 1) * 128, :],
                                  stage[:, :])

        # pool-path emission state
        import concourse.mybir as mb
        AluOp = mb.AluOpType

        def emit_pool_idx(h, c0, clen, idx_off):
            it = pidx_pool.tile([128, clen // 16], i16, tag="pidx")
            nc.sync.dma_start(it[:, :], pidxd[:, idx_off + c0 // 16:
                                               idx_off + (c0 + clen) // 16])
            pidx_tiles[(h, c0)] = it

        def emit_pool_chunk(h, c0, clen, vals_off, slab_state, idx_off):
            """Gather+scale+accumulate pool columns [c0, c0+clen) of hop h."""
            it = pidx_tiles.pop((h, c0))
            gt = pg_pool.tile([128, clen], u32, tag="pgath")
            nc.gpsimd.ap_gather(gt[:, :], xpair_sb[:, :], it[:, :],
                                channels=128, num_elems=NE, d=1,
                                num_idxs=clen)
            g16 = gt[:, :].bitcast(f16)  # [128, 2*clen]
            npieces_chunk = -(-2 * clen // 1024)
            for pi in range(npieces_chunk):
                plen = min(1024, 2 * clen - pi * 1024)
                vpsum = vps_pool.tile([128, 1024], f32)
                for half in range(-(-plen // 512)):
                    gcol = vals_off + 2 * c0 + pi * 1024 + half * 512
                    t = gcol // 2048
                    if slab_state.get("t") != t:
                        vs = vs_pool.tile([2, 2048], f16, tag="vband")
                        nc.sync.dma_start(vs[:, :],
                                          vals2d[:, 2048 * t:2048 * (t + 1)])
                        slab_state["t"] = t
                        slab_state["tile"] = vs
                    vs = slab_state["tile"]
                    k = gcol % 2048
                    hl = min(512, plen - half * 512)
                    nc.tensor.matmul(vpsum[:, half * 512:half * 512 + hl],
                                     ones2_sb[0:2, :], vs[:, k:k + hl],
                                     start=True, stop=True)
                o = pi * 1024
                if not globals().get("_SKIP_MULT"):
                    nc.vector.tensor_tensor(g16[:, o:o + plen],
                                            g16[:, o:o + plen],
                                            vpsum[:, 0:plen], AluOp.mult)
            # strided adds into acc
            if globals().get("_SKIP_ADDS"):
                return
            for (astart, a, b) in slab_state["pieces"](c0, c0 + clen):
                glo = a - c0
                nc.vector.tensor_tensor(
                    acc[:, 2 * astart:2 * astart + 2 * (b - a)],
                    acc[:, 2 * astart:2 * astart + 2 * (b - a)],
                    g16[:, 2 * glo:2 * glo + 2 * (b - a)], AluOp.add)

        def make_pieces(blocks):
            def pieces(lo, hi):
                out = []
                for (a, b) in blocks:
                    s, e = max(a, lo), min(b, hi)
                    if s < e:
                        out.append((s - a, s, e))
                return out
            return pieces

        # emission: hops sequential; interleave pool chunks among DMA groups
        mode = globals().get("_MODE", "full")
        vals_off = 0
        idx_off = 0
        for h in range(2):
            C = pool_cfg["C"][h]
            blocks = cfg["pool"]["blocks"][h]
            slab_state = {"t": None, "tile": None,
                          "pieces": make_pieces(blocks)}
            nc.vector.memset(acc[:, :], 0.0)
            chunk_bounds = []
            c0 = 0
            while c0 < C:
                clen = min(CH, C - c0)
                chunk_bounds.append((c0, clen))
                c0 += clen
            n_chunks = len(chunk_bounds)
            n_grps = len(grps)
            items = []
            ci = 0
            for gi, grp in enumerate(grps):
                while (mode != "dma-only"
                       and ci * n_grps < (gi + 1) * n_chunks
                       and ci < n_chunks):
                    items.append(("chunk", chunk_bounds[ci]))
                    ci += 1
                if mode != "pool-only":
                    items.append(("grp", grp))
            while mode != "dma-only" and ci < n_chunks:
                items.append(("chunk", chunk_bounds[ci]))
                ci += 1
            PF = 3
            def prefetch(item):
                if item[0] == "chunk":
                    emit_pool_idx(h, item[1][0], item[1][1], idx_off)
                else:
                    emit_dma_idx(h, item[1])
            for it_ in items[:PF]:
                prefetch(it_)
            for i, item in enumerate(items):
                if i + PF < len(items):
                    prefetch(items[i + PF])
                if item[0] == "chunk":
                    cs, cl = item[1]
                    emit_pool_chunk(h, cs, cl, vals_off, slab_state, idx_off)
                else:
                    emit_dma_group(h, item[1])
            # drain acc -> pout (host adds the two partition halves)
            nc.sync.dma_start(pouts[h][:, :], acc[:, :])
            vals_off += 2 * C
            idx_off += C // 16

    nc.finalize()
    return nc


def _run(inputs, trace=False, debug=False):
    from concourse.bass_utils import run_bass_kernel_spmd

    cfg, in_maps, perms_d, perms_p = _prep(**inputs)
    nc = _build(cfg, debug=debug)
    res = run_bass_kernel_spmd(nc, in_maps, core_ids=list(range(N_CORES)),
                               trace=trace)
    out = np.zeros((N_NODES, 2 * D_FEAT), dtype=np.float32)
    for c in range(N_CORES):
        for h in range(2):
            dev = res.results[c][f"out{h+1}"][:RPC, :]
            rows = c * RPC + perms_d[h][c]
            out[rows, h * D_FEAT:(h + 1) * D_FEAT] = dev
            pdev = np.asarray(res.results[c][f"pout{h+1}"], dtype=np.float32)
            pdev = (pdev[0:64] + pdev[64:128]).reshape(64, RPC, 2)
            pool_part = pdev.transpose(1, 0, 2).reshape(RPC, 128)
            rows_p = c * RPC + perms_p[h][c]
            out[rows_p, h * D_FEAT:(h + 1) * D_FEAT] += pool_part
    return out, res


def kernel(**inputs):
    out, _ = _run(inputs, trace=False)
    return out

